# revision 10
# baseline (speedup 1.0000x reference)
"""Trainium2 Bass kernel for DietConv2dV2: 3x3 conv (stride 1, pad 1) + bias.

x: (16, 8, 1024, 1024) fp32, weight: (8, 8, 3, 3), bias: (8,) -> out like x.

Strategy
--------
Data-parallel: 16 images / 8 cores = 2 images per core, no collectives.

Per core the conv runs as a banded matmul on the PE array:
  - K (contraction, partitions) = 16 input rows x 8 in-channels = 128,
    partition p = hi*8 + ci.
  - M (stationary free dim)     = 14 out rows x 8 out-channels = 112,
    column  m = ho*8 + co.
  - N (moving free dim)         = 512-wide w chunk.
The stationary "band" matrix S_kw[(hi,ci),(ho,co)] = weight[co,ci,hi-ho,kw]
covers all 3 kh taps at once; the 3 kw taps are 3 PSUM-accumulated matmuls
reading the same SBUF rows at w offsets kw-1 (rows stored with 1-col zero
padding on each side).  Band matrices are precomputed on the host from
`weight` (host preprocessing of a 2.3KB tensor) and loaded once.

The kernel is HBM-bandwidth bound (96.5% DMA busy at fp32), so all HBM
I/O is bf16: the host casts x to bf16 before upload, the PE does bf16
matmuls (fp32 PSUM accumulation), and the output is written as bf16 and
upcast to fp32 on the host.  This halves HBM traffic vs fp32
(432us -> ~230us) at ~6e-3 relative error, well inside the 2e-2 gate.
Bias is fused into the PSUM->SBUF eviction as a DVE per-partition
tensor_scalar add (which also performs the fp32->bf16 cast).
"""

import numpy as np
import ml_dtypes

import bass_rust
import concourse.bass as bass
import concourse.mybir as mybir
from concourse.tile import TileContext
from concourse.bass_utils import run_bass_kernel_spmd

F32 = mybir.dt.float32
BF16 = mybir.dt.bfloat16
NP_BF16 = np.dtype(ml_dtypes.bfloat16)

N_CORES = 8
IMG_PER_CORE = 2
C = 8          # channels (in == out)
H = 1024
W = 1024
KS = 3         # kernel size
HB = 14        # output rows per block (16 input rows -> 14 output rows)
KROWS = HB + KS - 1  # 16 input rows per block
M = C * HB     # 112 stationary columns
WCHUNK = 512   # PSUM bank = 512 fp32


def _split_excess_waits(nc):
    """This walrus build accepts 1 sync-wait per instruction (2 for
    EventSemaphore); Tile's final drain and ldweights can end up with
    more.  Move overflow waits onto EventSemaphore carriers inserted
    before the offender on the same engine."""
    for fn in nc.m.functions:
        for blk in fn.blocks:
            out = []
            changed = False
            for inst in blk.instructions:
                si = inst.sync_info
                cap = 2 if inst.opcode == "EventSemaphore" else 1
                waits = list(si.on_wait) if si is not None else []
                if len(waits) > cap:
                    changed = True
                    overflow, keep = waits[:-cap], waits[-cap:]
                    for j in range(0, len(overflow), 2):
                        es = mybir.InstEventSemaphore(
                            name=nc.get_next_instruction_name(), ins=[], outs=[]
                        )
                        es.engine = inst.engine
                        es.sync_info = bass_rust.SyncInfo(
                            on_wait=overflow[j : j + 2], on_update=[]
                        )
                        nc.register_instruction(es, overwrite=True)
                        out.append(es)
                    inst.sync_info = bass_rust.SyncInfo(
                        on_wait=keep, on_update=list(si.on_update)
                    )
                out.append(inst)
            if changed:
                blk.instructions = out


def _build(nimg, h, w, reps=1, salt=0):
    nblocks = -(-h // HB)
    nchunks = w // WCHUNK

    nc = bass.Bass(name=f"dietconv_s{salt}")
    x = nc.dram_tensor("x", [nimg, C, h, w], BF16, kind="ExternalInput")
    wb = nc.dram_tensor("wband", [KS, 128, M], BF16, kind="ExternalInput")
    bv = nc.dram_tensor("biasv", [M, 1], F32, kind="ExternalInput")
    out = nc.dram_tensor("out", [nimg, C, h, w], BF16, kind="ExternalOutput")

    # row-major (h, c) views so SBUF partition p = hi*8 + ci
    xr = x.rearrange("n c h w -> n h c w")
    outr = out.rearrange("n c h w -> n h c w")

    NXT = 10  # persistent input tiles (DMA prefetch depth)
    with TileContext(nc) as tc:
        with (
            tc.tile_pool(name="wpool", bufs=1) as wpool,
            tc.tile_pool(name="xpool", bufs=1) as xpool,
            tc.tile_pool(name="opool", bufs=6) as opool,
            tc.tile_pool(name="pspool", bufs=8, space="PSUM") as pspool,
        ):
            # weights/bias ride the HWDGE rings so the gpsimd (SWDGE)
            # queue is free to start streaming input tiles immediately
            wts = []
            for kw in range(KS):
                wt = wpool.tile([128, M], BF16, name=f"wt{kw}")
                nc.sync.dma_start(out=wt[:], in_=wb[kw])
                wts.append(wt)
            bt = wpool.tile([M, 1], F32, name="bt")
            nc.scalar.dma_start(out=bt[:], in_=bv[:])

            # Persistent x tiles: column c holds input w = c-1; cols 0 and
            # w+1 are zero padding for the edge taps.  The pad columns are
            # zeroed ONCE here (the per-block DMA only writes cols
            # 1..w+1), keeping the per-block DVE queue free of memsets --
            # at fp32 the in-loop memsets ahead of the PSUM evictions in
            # the strict-FIFO DVE queue stalled the PE ~2us every other
            # block (eviction convoy).
            xts = []
            for i in range(NXT):
                xt = xpool.tile([128, w + 2], BF16, name=f"xt{i}")
                nc.vector.memset(xt[:, 0:1], 0.0)
                nc.vector.memset(xt[:, w + 1 : w + 2], 0.0)
                xts.append(xt)

            def body(it0):
                it = it0
                for n in range(nimg):
                    for b in range(nblocks):
                        h0 = b * HB
                        nho = min(HB, h - h0)
                        hlo = h0 - 1  # input rows [hlo, hlo + KROWS)
                        vlo = max(hlo, 0)
                        vhi = min(hlo + KROWS, h)
                        plo = (vlo - hlo) * C
                        phi = (vhi - hlo) * C
                        xt = xts[it % NXT]
                        it += 1
                        # zero out-of-image rows (first/last block of each
                        # image only).  DVE partition start must be
                        # 32-aligned, so memset a wider aligned range; the
                        # DMA below rewrites the valid rows (Tile
                        # serializes the WAW overlap).  These memsets span
                        # the pad columns too but write zeros, so the
                        # prologue invariant holds.
                        if plo > 0:
                            nc.vector.memset(xt[0:plo, :], 0.0)
                        if phi < 128:
                            alo = (phi // 32) * 32
                            nc.vector.memset(xt[alo:128, :], 0.0)
                        # NOTE: replacing the 2-row HBM re-read (2/16 of
                        # input traffic) with an SBUF->SBUF halo copy from
                        # the previous block was tried twice (dedicated
                        # HWDGE ring included) and lost ~140us: Tile
                        # orders the copy against the same tile's main
                        # load, serializing the load pipeline on DMA
                        # completion latency.  The re-read overlaps
                        # freely and wins.
                        nc.gpsimd.dma_start(
                            out=xt[plo:phi, 1 : w + 1], in_=xr[n, vlo:vhi, :, :]
                        )
                        ot = opool.tile([M, w], BF16, name="ot", tag="ot")
                        # one PSUM tile (= one bank) per 512-wide w chunk
                        # (N=1024 matmuls fail the ISA s3d3_mm_num_elements
                        # check -- 512 is the hard cap), freed right after
                        # its own eviction; evictions alternate DVE / ACT
                        # so neither engine paces the PE (both also fuse
                        # the bias add + bf16 cast).  Each chunk's half of
                        # the output row DMAs out right after its own
                        # eviction (sync ring for j=0, scalar for j=1).
                        for j in range(nchunks):
                            base = j * WCHUNK
                            ps = pspool.tile([M, WCHUNK], F32, name="ps", tag="ps")
                            # kw tap reads tile col wo + kw (= input w + 1)
                            for kw in range(KS):
                                c0 = base + kw
                                nc.tensor.matmul(
                                    ps[:],
                                    wts[kw][:],
                                    xt[:, c0 : c0 + WCHUNK],
                                    start=(kw == 0),
                                    stop=(kw == KS - 1),
                                )
                            if j % 2 == 0:
                                nc.vector.tensor_scalar_add(
                                    ot[:, base : base + WCHUNK], ps[:], bt[:]
                                )
                                dma_eng = nc.sync
                            else:
                                nc.scalar.add(
                                    ot[:, base : base + WCHUNK], ps[:], bt[:]
                                )
                                dma_eng = nc.scalar
                            dma_eng.dma_start(
                                out=outr[n, h0 : h0 + nho, :, base : base + WCHUNK],
                                in_=ot[0 : nho * C, base : base + WCHUNK],
                            )
                return it

            # static unroll: tc.For_i loop control hits a walrus codegen
            # gap in this build ("ISA wrong length" on CompareAndBranch)
            it = 0
            for _ in range(reps):
                it = body(it)

    _split_excess_waits(nc)
    return nc


def _band_inputs(weight, bias):
    weight = np.asarray(weight, dtype=np.float32)
    bias = np.asarray(bias, dtype=np.float32)
    S = np.zeros((KS, 128, M), dtype=np.float32)
    for kw in range(KS):
        for kh in range(KS):
            blk = weight[:, :, kh, kw].T  # [ci, co]
            for ho in range(HB):
                hi = ho + kh
                S[kw, hi * C : (hi + 1) * C, ho * C : (ho + 1) * C] = blk
    biasv = np.tile(bias, HB).astype(np.float32)[:, None]  # m = ho*8 + co
    return S.astype(NP_BF16), biasv


def _in_maps(x, weight, bias, nimg_per_core, n_cores):
    S, biasv = _band_inputs(weight, bias)
    xh = np.asarray(x, dtype=np.float32).astype(NP_BF16)
    return [
        {
            "x": np.ascontiguousarray(xh[i * nimg_per_core : (i + 1) * nimg_per_core]),
            "wband": S,
            "biasv": biasv,
        }
        for i in range(n_cores)
    ]


def _run(x, weight, bias, nimg_per_core, h, w, n_cores, reps=1):
    in_maps = _in_maps(x, weight, bias, nimg_per_core, n_cores)
    # The walrus backend compile is rarely flaky (parallel codegen race).
    # jax caches the failed compilation by HLO, so retries must change the
    # BIR bytes (salt) and drop the jit cache.
    last_exc = None
    for attempt in range(4):
        try:
            nc = _build(nimg_per_core, h, w, reps, salt=attempt)
            res = run_bass_kernel_spmd(nc, in_maps, core_ids=list(range(n_cores)))
            break
        except Exception as e:  # noqa: BLE001
            last_exc = e
            try:
                import jax

                jax.clear_caches()
            except Exception:  # noqa: BLE001
                pass
    else:
        raise last_exc
    return np.concatenate([r["out"] for r in res.results], axis=0).astype(np.float32)


def kernel(x, weight, bias):
    return _run(x, weight, bias, IMG_PER_CORE, H, W, N_CORES, reps=1)


# revision 11
# speedup vs baseline: 1.1200x; 1.1200x over previous
"""Trainium2 Bass kernel for DietConv2dV2: 3x3 conv (stride 1, pad 1) + bias.

x: (16, 8, 1024, 1024) fp32, weight: (8, 8, 3, 3), bias: (8,) -> out like x.

Strategy
--------
Data-parallel: 16 images / 8 cores = 2 images per core, no collectives.

Per core the conv runs as a banded matmul on the PE array:
  - K (contraction, partitions) = 16 input rows x 8 in-channels = 128,
    partition p = hi*8 + ci.
  - M (stationary free dim)     = 14 out rows x 8 out-channels = 112,
    column  m = ho*8 + co.
  - N (moving free dim)         = 512-wide w chunk.
The stationary "band" matrix S_kw[(hi,ci),(ho,co)] = weight[co,ci,hi-ho,kw]
covers all 3 kh taps at once; the 3 kw taps are 3 PSUM-accumulated matmuls
reading the same SBUF rows at w offsets kw-1 (rows stored with 1-col zero
padding on each side).  Band matrices are precomputed on the host from
`weight` (host preprocessing of a 2.3KB tensor) and loaded once.

The kernel is HBM-bandwidth bound (96.5% DMA busy at fp32), so all HBM
I/O is bf16: the host casts x to bf16 before upload, the PE does bf16
matmuls (fp32 PSUM accumulation), and the output is written as bf16 and
upcast to fp32 on the host.  This halves HBM traffic vs fp32
(432us -> ~230us) at ~6e-3 relative error, well inside the 2e-2 gate.
Bias is fused into the PSUM->SBUF eviction as a DVE per-partition
tensor_scalar add (which also performs the fp32->bf16 cast).
"""

import numpy as np
import ml_dtypes

import bass_rust
import concourse.bass as bass
import concourse.mybir as mybir
from concourse.tile import TileContext
from concourse.bass_utils import run_bass_kernel_spmd

F32 = mybir.dt.float32
BF16 = mybir.dt.bfloat16
NP_BF16 = np.dtype(ml_dtypes.bfloat16)

N_CORES = 8
IMG_PER_CORE = 2
C = 8          # channels (in == out)
H = 1024
W = 1024
KS = 3         # kernel size
HB = 14        # output rows per block (16 input rows -> 14 output rows)
KROWS = HB + KS - 1  # 16 input rows per block
M = C * HB     # 112 stationary columns
WCHUNK = 512   # PSUM bank = 512 fp32


def _split_excess_waits(nc):
    """This walrus build accepts 1 sync-wait per instruction (2 for
    EventSemaphore); Tile's final drain and ldweights can end up with
    more.  Move overflow waits onto EventSemaphore carriers inserted
    before the offender on the same engine."""
    for fn in nc.m.functions:
        for blk in fn.blocks:
            out = []
            changed = False
            for inst in blk.instructions:
                si = inst.sync_info
                cap = 2 if inst.opcode == "EventSemaphore" else 1
                waits = list(si.on_wait) if si is not None else []
                if len(waits) > cap:
                    changed = True
                    overflow, keep = waits[:-cap], waits[-cap:]
                    for j in range(0, len(overflow), 2):
                        es = mybir.InstEventSemaphore(
                            name=nc.get_next_instruction_name(), ins=[], outs=[]
                        )
                        es.engine = inst.engine
                        es.sync_info = bass_rust.SyncInfo(
                            on_wait=overflow[j : j + 2], on_update=[]
                        )
                        nc.register_instruction(es, overwrite=True)
                        out.append(es)
                    inst.sync_info = bass_rust.SyncInfo(
                        on_wait=keep, on_update=list(si.on_update)
                    )
                out.append(inst)
            if changed:
                blk.instructions = out


def _build(nimg, h, w, reps=1, salt=0):
    nblocks = -(-h // HB)
    nchunks = w // WCHUNK

    nc = bass.Bass(name=f"dietconv_s{salt}")
    x = nc.dram_tensor("x", [nimg, C, h, w], BF16, kind="ExternalInput")
    wb = nc.dram_tensor("wband", [KS, 128, M], BF16, kind="ExternalInput")
    bv = nc.dram_tensor("biasv", [M, 1], F32, kind="ExternalInput")
    out = nc.dram_tensor("out", [nimg, C, h, w], BF16, kind="ExternalOutput")

    # row-major (h, c) views so SBUF partition p = hi*8 + ci
    xr = x.rearrange("n c h w -> n h c w")
    outr = out.rearrange("n c h w -> n h c w")

    NXT = 10  # persistent input tiles (DMA prefetch depth)
    with TileContext(nc) as tc:
        with (
            tc.tile_pool(name="wpool", bufs=1) as wpool,
            tc.tile_pool(name="xpool", bufs=1) as xpool,
            tc.tile_pool(name="opool", bufs=6) as opool,
            tc.tile_pool(name="pspool", bufs=8, space="PSUM") as pspool,
        ):
            # weights/bias ride the HWDGE rings so the gpsimd (SWDGE)
            # queue is free to start streaming input tiles immediately
            wts = []
            for kw in range(KS):
                wt = wpool.tile([128, M], BF16, name=f"wt{kw}")
                nc.sync.dma_start(out=wt[:], in_=wb[kw])
                wts.append(wt)
            bt = wpool.tile([M, 1], F32, name="bt")
            nc.scalar.dma_start(out=bt[:], in_=bv[:])

            # Persistent x tiles: column c holds input w = c-1; cols 0 and
            # w+1 are zero padding for the edge taps.  The pad columns are
            # zeroed ONCE here (the per-block DMA only writes cols
            # 1..w+1), keeping the per-block DVE queue free of memsets --
            # at fp32 the in-loop memsets ahead of the PSUM evictions in
            # the strict-FIFO DVE queue stalled the PE ~2us every other
            # block (eviction convoy).
            xts = []
            for i in range(NXT):
                xt = xpool.tile([128, w + 2], BF16, name=f"xt{i}")
                nc.vector.memset(xt[:, 0:1], 0.0)
                nc.vector.memset(xt[:, w + 1 : w + 2], 0.0)
                xts.append(xt)

            def body(it0):
                it = it0
                for n in range(nimg):
                    for b in range(nblocks):
                        h0 = b * HB
                        nho = min(HB, h - h0)
                        hlo = h0 - 1  # input rows [hlo, hlo + KROWS)
                        vlo = max(hlo, 0)
                        vhi = min(hlo + KROWS, h)
                        plo = (vlo - hlo) * C
                        phi = (vhi - hlo) * C
                        xt = xts[it % NXT]
                        it += 1
                        # zero out-of-image rows (first/last block of each
                        # image only).  DVE partition start must be
                        # 32-aligned, so memset a wider aligned range; the
                        # DMA below rewrites the valid rows (Tile
                        # serializes the WAW overlap).  These memsets span
                        # the pad columns too but write zeros, so the
                        # prologue invariant holds.
                        if plo > 0:
                            nc.vector.memset(xt[0:plo, :], 0.0)
                        if phi < 128:
                            alo = (phi // 32) * 32
                            nc.vector.memset(xt[alo:128, :], 0.0)
                        # NOTE: replacing the 2-row HBM re-read (2/16 of
                        # input traffic) with an SBUF->SBUF halo copy from
                        # the previous block was tried twice (dedicated
                        # HWDGE ring included) and lost ~140us: Tile
                        # orders the copy against the same tile's main
                        # load, serializing the load pipeline on DMA
                        # completion latency.  The re-read overlaps
                        # freely and wins.
                        nc.gpsimd.dma_start(
                            out=xt[plo:phi, 1 : w + 1], in_=xr[n, vlo:vhi, :, :]
                        )
                        ot = opool.tile([M, w], BF16, name="ot", tag="ot")
                        # one PSUM tile (= one bank) per 512-wide w chunk
                        # (N=1024 matmuls fail the ISA s3d3_mm_num_elements
                        # check -- 512 is the hard cap), freed right after
                        # its own eviction; evictions alternate DVE / ACT
                        # so neither engine paces the PE (both also fuse
                        # the bias add + bf16 cast).  Output DMA stays one
                        # whole-block transfer: per-chunk half-DMAs were
                        # tried and lost 27us -- the extra ~840ns
                        # DMA_DIRECT2D issues on the scalar queue delay
                        # the ACT evictions, stalling the PE on PSUM.
                        for j in range(nchunks):
                            base = j * WCHUNK
                            ps = pspool.tile([M, WCHUNK], F32, name="ps", tag="ps")
                            # kw tap reads tile col wo + kw (= input w + 1)
                            for kw in range(KS):
                                c0 = base + kw
                                nc.tensor.matmul(
                                    ps[:],
                                    wts[kw][:],
                                    xt[:, c0 : c0 + WCHUNK],
                                    start=(kw == 0),
                                    stop=(kw == KS - 1),
                                )
                            if j % 2 == 0:
                                nc.vector.tensor_scalar_add(
                                    ot[:, base : base + WCHUNK], ps[:], bt[:]
                                )
                            else:
                                nc.scalar.add(
                                    ot[:, base : base + WCHUNK], ps[:], bt[:]
                                )
                        # alternate output DMAs across both HWDGE rings
                        # (sync + scalar): ~70us faster than one ring
                        dma_eng = nc.sync if b % 2 == 0 else nc.scalar
                        dma_eng.dma_start(
                            out=outr[n, h0 : h0 + nho, :, :],
                            in_=ot[0 : nho * C, :],
                        )
                return it

            # static unroll: tc.For_i loop control hits a walrus codegen
            # gap in this build ("ISA wrong length" on CompareAndBranch)
            it = 0
            for _ in range(reps):
                it = body(it)

    _split_excess_waits(nc)
    return nc


def _band_inputs(weight, bias):
    weight = np.asarray(weight, dtype=np.float32)
    bias = np.asarray(bias, dtype=np.float32)
    S = np.zeros((KS, 128, M), dtype=np.float32)
    for kw in range(KS):
        for kh in range(KS):
            blk = weight[:, :, kh, kw].T  # [ci, co]
            for ho in range(HB):
                hi = ho + kh
                S[kw, hi * C : (hi + 1) * C, ho * C : (ho + 1) * C] = blk
    biasv = np.tile(bias, HB).astype(np.float32)[:, None]  # m = ho*8 + co
    return S.astype(NP_BF16), biasv


def _in_maps(x, weight, bias, nimg_per_core, n_cores):
    S, biasv = _band_inputs(weight, bias)
    xh = np.asarray(x, dtype=np.float32).astype(NP_BF16)
    return [
        {
            "x": np.ascontiguousarray(xh[i * nimg_per_core : (i + 1) * nimg_per_core]),
            "wband": S,
            "biasv": biasv,
        }
        for i in range(n_cores)
    ]


def _run(x, weight, bias, nimg_per_core, h, w, n_cores, reps=1):
    in_maps = _in_maps(x, weight, bias, nimg_per_core, n_cores)
    # The walrus backend compile is rarely flaky (parallel codegen race).
    # jax caches the failed compilation by HLO, so retries must change the
    # BIR bytes (salt) and drop the jit cache.
    last_exc = None
    for attempt in range(4):
        try:
            nc = _build(nimg_per_core, h, w, reps, salt=attempt)
            res = run_bass_kernel_spmd(nc, in_maps, core_ids=list(range(n_cores)))
            break
        except Exception as e:  # noqa: BLE001
            last_exc = e
            try:
                import jax

                jax.clear_caches()
            except Exception:  # noqa: BLE001
                pass
    else:
        raise last_exc
    return np.concatenate([r["out"] for r in res.results], axis=0).astype(np.float32)


def kernel(x, weight, bias):
    return _run(x, weight, bias, IMG_PER_CORE, H, W, N_CORES, reps=1)
